# revision 7
# baseline (speedup 1.0000x reference)
"""Trainium2 Bass kernel for nn_AllAtomDecoder (gnn_message_passing).

Math: all 34 side-chain atom slots of residue i are placed at CA_i, so the
[A,A] (A = L*34) radius-graph adjacency is a residue-level [L,L] adjacency
R expanded by per-atom validity vm:
    adj[(i,s),(j,t)] = R[i,j] * vm[i,s] * vm[j,t] * (1 - delta_{(i,s),(j,t)})
with R[i,i] = 1 (distance 0 < 8).  Hence
    msg[(i,s),:] = vm[i,s] * (M[i,:] - remb[i,:] - atom_sc[s,:])
where S[j,:] = cnt_j * remb[j,:] + vm[j,:] @ atom_sc   (cnt_j = sum_t vm[j,t])
      M     = R @ S                                    ([L,L] @ [L,D])

Sharding: 8 cores; cores 0-3 own batch 0, cores 4-7 batch 1; each core
computes the residue-level stages for its batch and emits 32 residues
([32, 34*128] f32) of the final output.

Implementation: raw bacc (no TileContext) with hand-placed semaphores.
The per-core output [32 res, 34 t, 128 d] is computed with the t-axis
split into 4 groups (9,9,8,8) packed onto the partition axis:
partition p = 32*tg + l, so the two big DVE ops run ~1.2k columns on
all 128 partitions.  All small inputs ride in one packed [128, 748]
tensor, DMA'd as two halves on the two HWDGE rings (sync + scalar).
"""

from contextlib import ExitStack

import numpy as np

import concourse.bass as bass
import concourse.bacc as bacc
import concourse.mybir as mybir
from concourse.bass_utils import run_bass_kernel_spmd

F32 = mybir.dt.float32
ALU = mybir.AluOpType
AX = mybir.AxisListType

B = 2
L = 128          # residues per batch
NCLS = 20        # enabled residue classes (>=20 are argmax-disabled)
NSC = 34         # side-chain atom slots
D = 128          # embedding dim
RPC = 32         # residues per core
NCORES = 8
R2 = 64.0        # RADIUS**2

TB = [0, 9, 18, 26]   # t-group bases
TW = [9, 9, 8, 8]     # t-group widths

# pack column offsets
_off = {}
_c = 0
for _name, _w in [("aa_f", NCLS), ("remb_f", D), ("cat_f", L), ("maskv", 1),
                  ("tbl", NSC), ("atom", D), ("aa_o4", NCLS), ("remb_o4", D),
                  ("cat_o", RPC), ("mask_o4", 1), ("eye", L)]:
    _off[_name] = _c
    _c += _w
PACKW = _c  # 748


def build_nc():
    """Build the SPMD per-core Bass graph (identical on all 8 cores)."""
    nc = bacc.Bacc("TRN2", target_bir_lowering=False, debug=False,
                   num_devices=NCORES)

    pack = nc.dram_tensor("pack", [L, PACKW], F32, kind="ExternalInput")
    atom = nc.dram_tensor("atom", [NSC, D], F32, kind="ExternalInput")
    out = nc.dram_tensor("out", [RPC, NSC * D], F32, kind="ExternalOutput")
    out3 = out[:].rearrange("l (t d) -> l t d", d=D)
    aflat = atom[:].rearrange("t d -> (t d)")

    with ExitStack() as ctx:
        e = ctx.enter_context

        # ---------------- SBUF ----------------
        pk = e(nc.sbuf_tensor([L, PACKW], F32))
        ones3 = e(nc.sbuf_tensor([3, L], F32))
        oh_all = e(nc.sbuf_tensor([L, 64], F32))
        catsq_f = e(nc.sbuf_tensor([3, L], F32))
        catm2_f = e(nc.sbuf_tensor([3, L], F32))
        catsq_o = e(nc.sbuf_tensor([3, RPC], F32))
        tblsum = e(nc.sbuf_tensor([NCLS, 1], F32))
        rmax_f = e(nc.sbuf_tensor([L, 1], F32))
        rmax_o = e(nc.sbuf_tensor([L, 1], F32))
        sqT_f = e(nc.sbuf_tensor([1, L], F32))
        sqT_o = e(nc.sbuf_tensor([1, RPC], F32))
        ohT_f = e(nc.sbuf_tensor([RPC, L], F32))
        ohT_o4 = e(nc.sbuf_tensor([RPC, L], F32))
        scmT = e(nc.sbuf_tensor([NSC, L], F32))
        S_t = e(nc.sbuf_tensor([L, D], F32))
        rcols4 = e(nc.sbuf_tensor([L, L], F32))
        q4 = e(nc.sbuf_tensor([L, D], F32))
        atom_rep = e(nc.sbuf_tensor([L, 9, D], F32))
        v4 = e(nc.sbuf_tensor([L, 9, D], F32))
        o4 = e(nc.sbuf_tensor([L, 9, D], F32))

        # views into the pack
        aa_f_t = pk[:, _off["aa_f"]:_off["aa_f"] + NCLS]
        remb_f_t = pk[:, _off["remb_f"]:_off["remb_f"] + D]
        cat_f_t = pk[:3, _off["cat_f"]:_off["cat_f"] + L]
        maskv_t = pk[:, _off["maskv"]:_off["maskv"] + 1]
        tbl_t = pk[:NCLS, _off["tbl"]:_off["tbl"] + NSC]
        atom_t = pk[:NSC, _off["atom"]:_off["atom"] + D]
        aa_o4_t = pk[:, _off["aa_o4"]:_off["aa_o4"] + NCLS]
        remb_o4_t = pk[:, _off["remb_o4"]:_off["remb_o4"] + D]
        cat_o_t = pk[:3, _off["cat_o"]:_off["cat_o"] + RPC]
        mask_o4_t = pk[:, _off["mask_o4"]:_off["mask_o4"] + 1]
        eye_t = pk[:, _off["eye"]:_off["eye"] + L]

        # ---------------- PSUM (8 banks) ----------------
        ohT_f_p = e(nc.psum_tensor([RPC, L], F32))       # b0
        ohT_o4_p = e(nc.psum_tensor([RPC, L], F32))      # b1
        scmT_p = e(nc.psum_tensor([L, D], F32))          # b2 (reused for m4)
        temb_p = e(nc.psum_tensor([L, D], F32))          # b3
        sq_p = e(nc.psum_tensor([1, L + RPC], F32))      # b4 (sqT_f | sqT_o)
        cnt_p = e(nc.psum_tensor([L, 1], F32))           # b5
        scm4_p = e(nc.psum_tensor([L, 9], F32))          # b6
        d2_p = e(nc.psum_tensor([L, RPC], F32))          # b7
        m4_p = scmT_p  # bank reuse: scmT consumed before m4 is written

        sem_in = e(nc.semaphore("sem_in"))
        sem_atom = e(nc.semaphore("sem_atom"))
        sem_dve = e(nc.semaphore("sem_dve"))
        sem_pe = e(nc.semaphore("sem_pe"))
        sem_out = e(nc.semaphore("sem_out"))

        block = e(nc.Block())

        # ---------------- DMA ring 1: sync ----------------
        @block.sync
        def _(eng):
            eng.dma_start(pk[:64, :], pack[:64, :]).then_inc(sem_in, 16)
            eng.wait_ge(sem_dve, 21)            # mul_a done -> tg2/tg3 ready
            for tg in (2, 3):
                eng.dma_start(
                    out3[:, TB[tg]:TB[tg] + TW[tg], :],
                    o4[32 * tg:32 * (tg + 1), :TW[tg], :],
                ).then_inc(sem_out, 16)
            eng.wait_ge(sem_out, 64)            # all output landed

        # ---------------- DMA ring 2: scalar ----------------
        @block.scalar
        def _(eng):
            eng.dma_start(pk[64:, :], pack[64:, :]).then_inc(sem_in, 16)
            for tg in range(4):
                tb, tw = TB[tg], TW[tg]
                eng.dma_start(
                    atom_rep[32 * tg:32 * (tg + 1), :tw, :]
                    .rearrange("l t d -> l (t d)"),
                    aflat[tb * D:(tb + tw) * D][None, :]
                    .to_broadcast((RPC, tw * D)),
                ).then_inc(sem_atom, 16)
            eng.wait_ge(sem_dve, 22)            # mul_b done -> tg0/tg1 ready
            for tg in (0, 1):
                eng.dma_start(
                    out3[:, TB[tg]:TB[tg] + TW[tg], :],
                    o4[32 * tg:32 * (tg + 1), :TW[tg], :],
                ).then_inc(sem_out, 16)

        # ---------------- DVE ----------------
        @block.vector
        def _(eng):
            v = nc.vector
            v.memset(ones3[:], 1.0).then_inc(sem_dve, 1)            # 1
            v.memset(oh_all[:], 0.0).then_inc(sem_dve, 1)           # 2
            eng.wait_ge(sem_in, 32)
            v.tensor_tensor(catsq_f[:], cat_f_t, cat_f_t,
                            op=ALU.mult).then_inc(sem_dve, 1)       # 3
            v.tensor_tensor(catsq_o[:], cat_o_t, cat_o_t,
                            op=ALU.mult).then_inc(sem_dve, 1)       # 4
            v.tensor_scalar(catm2_f[:], cat_f_t, -2.0, None,
                            ALU.mult).then_inc(sem_dve, 1)          # 5
            v.tensor_reduce(tblsum[:], tbl_t, op=ALU.add,
                            axis=AX.X).then_inc(sem_dve, 1)         # 6
            v.tensor_reduce(rmax_f[:], aa_f_t, op=ALU.max,
                            axis=AX.X).then_inc(sem_dve, 1)         # 7
            eng.wait_ge(sem_dve, 7)
            v.tensor_scalar(oh_all[:, :NCLS], aa_f_t, rmax_f[:, :1],
                            maskv_t, ALU.is_ge,
                            ALU.mult).then_inc(sem_dve, 1)          # 8
            v.tensor_reduce(rmax_o[:], aa_o4_t, op=ALU.max,
                            axis=AX.X).then_inc(sem_dve, 1)         # 9
            eng.wait_ge(sem_dve, 9)
            v.tensor_scalar(oh_all[:, 32:32 + NCLS], aa_o4_t,
                            rmax_o[:, :1], mask_o4_t, ALU.is_ge,
                            ALU.mult).then_inc(sem_dve, 1)          # 10
            eng.wait_ge(sem_pe, 2)              # sq matmuls done
            v.tensor_copy(sqT_f[:], sq_p[:1, :L]).then_inc(sem_dve, 1)   # 11
            v.tensor_copy(sqT_o[:], sq_p[:1, L:]).then_inc(sem_dve, 1)   # 12
            eng.wait_ge(sem_pe, 4)              # transposes done
            v.tensor_copy(ohT_f[:], ohT_f_p[:]).then_inc(sem_dve, 1)     # 13
            v.tensor_copy(ohT_o4[:], ohT_o4_p[:]).then_inc(sem_dve, 1)   # 14
            eng.wait_ge(sem_pe, 5)              # scmT done
            v.tensor_copy(scmT[:], scmT_p[:NSC, :]).then_inc(sem_dve, 1)  # 15
            eng.wait_ge(sem_pe, 7)              # cnt + temb done
            v.scalar_tensor_tensor(S_t[:], remb_f_t, cnt_p[:, :1],
                                   temb_p[:], ALU.mult,
                                   ALU.add).then_inc(sem_dve, 1)         # 16
            eng.wait_ge(sem_pe, 14)             # d2 group done
            v.tensor_scalar(
                rcols4[:].rearrange("j (a b) -> j a b", b=RPC),
                d2_p[:, None, :].to_broadcast((L, 4, RPC)),
                R2, None, ALU.is_lt).then_inc(sem_dve, 1)                # 17
            eng.wait_ge(sem_pe, 15)             # m4 done
            v.tensor_tensor(q4[:], m4_p[:], remb_o4_t,
                            op=ALU.subtract).then_inc(sem_dve, 1)        # 18
            eng.wait_ge(sem_atom, 64)
            eng.wait_ge(sem_dve, 18)
            v.tensor_tensor(
                v4[:, :8, :], q4[:, None, :].to_broadcast((L, 8, D)),
                atom_rep[:, :8, :],
                op=ALU.subtract).then_inc(sem_dve, 1)                    # 19
            v.tensor_tensor(
                v4[:64, 8:9, :], q4[:64, None, :].to_broadcast((64, 1, D)),
                atom_rep[:64, 8:9, :],
                op=ALU.subtract).then_inc(sem_dve, 1)                    # 20
            eng.wait_ge(sem_dve, 20)
            v.tensor_tensor(
                o4[:, :8, :], v4[:, :8, :],
                scm4_p[:, :8, None].to_broadcast((L, 8, D)),
                op=ALU.mult).then_inc(sem_dve, 1)                        # 21
            v.tensor_tensor(
                o4[:64, 8:9, :], v4[:64, 8:9, :],
                scm4_p[:64, 8:9, None].to_broadcast((64, 1, D)),
                op=ALU.mult).then_inc(sem_dve, 1)                        # 22

        # ---------------- PE ----------------
        @block.tensor
        def _(eng):
            t = nc.tensor
            eng.wait_ge(sem_dve, 4)             # ones3 + catsq ready
            t.matmul(sq_p[:1, :L], ones3[:, :1],
                     catsq_f[:]).then_inc(sem_pe, 1)                # 1
            t.matmul(sq_p[:1, L:], ones3[:, :1],
                     catsq_o[:]).then_inc(sem_pe, 1)                # 2
            eng.wait_ge(sem_dve, 10)            # one-hots written
            t.transpose(ohT_f_p[:], oh_all[:, :RPC],
                        eye_t).then_inc(sem_pe, 1)                  # 3
            t.transpose(ohT_o4_p[:], oh_all[:, RPC:],
                        eye_t).then_inc(sem_pe, 1)                  # 4
            eng.wait_ge(sem_dve, 14)            # ohT copies done
            t.matmul(scmT_p[:NSC, :], tbl_t,
                     ohT_f[:NCLS, :]).then_inc(sem_pe, 1)           # 5
            t.matmul(cnt_p[:], ohT_f[:NCLS, :],
                     tblsum[:]).then_inc(sem_pe, 1)                 # 6
            eng.wait_ge(sem_dve, 15)            # scmT copy done
            t.matmul(temb_p[:], scmT[:], atom_t).then_inc(sem_pe, 1)  # 7
            for tg in range(4):
                tb, tw = TB[tg], TW[tg]
                t.matmul(scm4_p[32 * tg:32 * (tg + 1), :tw],
                         ohT_o4[:NCLS, 32 * tg:32 * (tg + 1)],
                         tbl_t[:, tb:tb + tw],
                         tile_position=(0, 32 * tg),
                         ).then_inc(sem_pe, 1)                      # 8-11
            t.matmul(d2_p[:], catm2_f[:], cat_o_t,
                     start=True, stop=False).then_inc(sem_pe, 1)    # 12
            t.matmul(d2_p[:], sqT_f[:], ones3[:1, :RPC],
                     start=False, stop=False).then_inc(sem_pe, 1)   # 13
            t.matmul(d2_p[:], ones3[:1, :L], sqT_o[:],
                     start=False, stop=True).then_inc(sem_pe, 1)    # 14
            eng.wait_ge(sem_dve, 17)            # rcols4 (and S) ready
            t.matmul(m4_p[:], rcols4[:], S_t[:]).then_inc(sem_pe, 1)  # 15

    nc.compile()
    return nc


def make_in_maps(aa_pred, residue_embeddings, bb_pred, mask,
                 valid_atom37_mask, atom_embed):
    f32 = lambda x: np.ascontiguousarray(x, dtype=np.float32)
    eye = np.eye(L, dtype=np.float32)
    in_maps = []
    for c in range(NCORES):
        b = c // (NCORES // B)
        r0 = (c % (NCORES // B)) * RPC
        pk = np.zeros((L, PACKW), dtype=np.float32)

        def put(name, arr):
            arr = f32(arr)
            pk[:arr.shape[0], _off[name]:_off[name] + arr.shape[1]] = arr

        put("aa_f", aa_pred[b, :, :NCLS])
        put("remb_f", residue_embeddings[b])
        put("cat_f", bb_pred[b, :, 1, :].T)
        put("maskv", mask[b][:, None])
        put("tbl", valid_atom37_mask[:NCLS, 3:])
        put("atom", atom_embed[3:])
        put("aa_o4", np.tile(aa_pred[b, r0:r0 + RPC, :NCLS], (4, 1)))
        put("remb_o4", np.tile(residue_embeddings[b, r0:r0 + RPC], (4, 1)))
        put("cat_o", bb_pred[b, r0:r0 + RPC, 1, :].T)
        put("mask_o4", np.tile(mask[b, r0:r0 + RPC][:, None], (4, 1)))
        put("eye", eye)
        in_maps.append({"pack": pk, "atom": f32(atom_embed[3:])})
    return in_maps


def gather_out(results):
    chunks = [np.asarray(r["out"]).reshape(RPC, NSC, D) for r in results]
    full = np.concatenate(chunks, axis=0)          # [256, 34, 128]
    return full.reshape(B, L * NSC, D)


def kernel(**inputs) -> np.ndarray:
    nc = build_nc()
    in_maps = make_in_maps(**inputs)
    res = run_bass_kernel_spmd(nc, in_maps, core_ids=list(range(NCORES)))
    return gather_out(res.results)
